# revision 15
# baseline (speedup 1.0000x reference)
"""Trainium2 Bass kernel for nn_DetectNet (nms_detection).

Sharding: data-parallel over row-bands — core d owns frame rows
[256d, 256(d+1)). The 64-iteration connected-components label propagation
runs fully on-device on all 8 NeuronCores with 64-row halos per side
(zero cross-core communication: error from the halo edge travels 1 row per
iteration, so owned rows stay exact).

Per-core CC layout: [128 partitions, 6144 free], partition p holds frame
columns [16p, 16p+16), free index = c_local*384 + r (column strips of 16,
row-major within strip). Row-neighbor min = free-offset +-1; column-neighbor
min = free-offset +-384 (within partition) plus a cross-partition shift of the
boundary strips done on the TensorEngine via a 128x128 shift matrix.
"""
import numpy as np

import concourse.bass as bass
import concourse.mybir as mybir
from concourse.bass_utils import run_bass_kernel_spmd

H, W = 2048, 2048
CS = 227
K = 16
NC_CLS = 4
MIN_SIZE = 64
CC_ITERS = 64
BIG = float(H * W)

NCORES = 8
OWN = H // NCORES          # 256 rows owned per core
HALO = 64
EXT = OWN + 2 * HALO       # 384 rows in CC array
CSTRIP = 16                # columns per partition
FREE = CSTRIP * EXT        # 6144


# ----------------------------------------------------------------------------
# device program: 64 CC iterations (min-label propagation, 4-connectivity)
# ----------------------------------------------------------------------------
_CC_CACHE = {}


def _build_cc_program():
    if "nc" in _CC_CACHE:
        return _CC_CACHE["nc"]
    dt = mybir.dt.float32
    nc = bass.Bass()
    lab_in = nc.dram_tensor("lab0", [128, FREE], dt, kind="ExternalInput")
    mb_in = nc.dram_tensor("mb", [128, FREE], dt, kind="ExternalInput")
    # shift matrices: Sdn[k, m] = 1 if k == m-1 ; Sup[k, m] = 1 if k == m+1
    sdn_in = nc.dram_tensor("sdn", [128, 128], dt, kind="ExternalInput")
    sup_in = nc.dram_tensor("sup", [128, 128], dt, kind="ExternalInput")
    # per-partition bias vectors: BIG at the frame-edge partition, else 0
    bl_in = nc.dram_tensor("biasl", [128, 1], dt, kind="ExternalInput")
    br_in = nc.dram_tensor("biasr", [128, 1], dt, kind="ExternalInput")
    lab_out = nc.dram_tensor("lab_out", [128, CSTRIP * OWN], dt,
                             kind="ExternalOutput")

    with (
        nc.sbuf_tensor([128, FREE], dt) as lab,
        nc.sbuf_tensor([128, FREE], dt) as mb,
        nc.sbuf_tensor([128, FREE], dt) as t1,
        nc.sbuf_tensor([128, FREE], dt) as t2,
        nc.sbuf_tensor([128, 128], dt) as sdn,
        nc.sbuf_tensor([128, 128], dt) as sup,
        nc.sbuf_tensor([128, 1], dt) as bl,
        nc.sbuf_tensor([128, 1], dt) as br,
        nc.sbuf_tensor([128, EXT], dt) as lb,       # left-neighbor strip
        nc.sbuf_tensor([128, EXT], dt) as rb,       # right-neighbor strip
        nc.psum_tensor([128, EXT], dt) as psl,
        nc.psum_tensor([128, EXT], dt) as psr,
        nc.semaphore("dma_sem") as dma_sem,
        nc.semaphore("pe_sem") as pe_sem,
        nc.semaphore("act_sem") as act_sem,
        nc.semaphore("dve_sem") as dve_sem,
        nc.Block() as block,
    ):
        mn = mybir.AluOpType.min
        mx = mybir.AluOpType.max
        DVE_PER = 6   # dve sem increments per iteration
        LAST = EXT * (CSTRIP - 1)  # free offset of last column strip = 5760

        @block.sync
        def _(sync):
            sync.dma_start(out=lab[:], in_=lab_in[:]).then_inc(dma_sem, 16)
            sync.dma_start(out=mb[:], in_=mb_in[:]).then_inc(dma_sem, 16)
            sync.dma_start(out=sdn[:], in_=sdn_in[:]).then_inc(dma_sem, 16)
            sync.dma_start(out=sup[:], in_=sup_in[:]).then_inc(dma_sem, 16)
            sync.dma_start(out=bl[:], in_=bl_in[:]).then_inc(dma_sem, 16)
            sync.dma_start(out=br[:], in_=br_in[:]).then_inc(dma_sem, 16)
            # wait for the full CC loop to finish, then write owned rows out
            sync.wait_ge(dve_sem, CC_ITERS * DVE_PER)
            src = lab[:].rearrange("p (c r) -> p c r", r=EXT)[:, :, HALO:HALO + OWN]
            dst = lab_out[:].rearrange("p (c r) -> p c r", r=OWN)
            sync.dma_start(out=dst, in_=src).then_inc(dma_sem, 16)

        @block.tensor
        def _(tensor):
            tensor.wait_ge(dma_sem, 96)
            for i in range(CC_ITERS):
                # wait for lab from previous iteration (op 8 of iter i)
                if i > 0:
                    tensor.wait_ge(dve_sem, i * DVE_PER)
                # psl[p] = lab[p-1, last strip]  (left neighbor of col strip 0)
                nc.tensor.matmul(out=psl[:], lhsT=sdn[:],
                                 rhs=lab[:, LAST:LAST + EXT],
                                 start=True, stop=True).then_inc(pe_sem, 1)
                # psr[p] = lab[p+1, strip 0]  (right neighbor of last strip)
                nc.tensor.matmul(out=psr[:], lhsT=sup[:],
                                 rhs=lab[:, 0:EXT],
                                 start=True, stop=True).then_inc(pe_sem, 1)

        @block.scalar
        def _(scalar):
            cp = mybir.ActivationFunctionType.Identity
            scalar.wait_ge(dma_sem, 96)
            for i in range(CC_ITERS):
                scalar.wait_ge(pe_sem, i * 2 + 2)
                # copy PSUM->SBUF, adding BIG on the frame-edge partition so
                # the shifted-in "neighbor" is neutral in the min
                nc.scalar.activation(out=lb[:], in_=psl[:], func=cp,
                                     bias=bl[:, 0:1]).then_inc(act_sem, 1)
                nc.scalar.activation(out=rb[:], in_=psr[:], func=cp,
                                     bias=br[:, 0:1]).then_inc(act_sem, 1)

        @block.vector
        def _(vector):
            vector.wait_ge(dma_sem, 96)
            for i in range(CC_ITERS):
                vector.wait_ge(act_sem, i * 2 + 2)
                # t1 = min(up, down) along rows (free offset +-1)
                nc.vector.tensor_tensor(
                    out=t1[:, 1:FREE - 1], in0=lab[:, 0:FREE - 2],
                    in1=lab[:, 2:FREE], op=mn).then_inc(dve_sem, 1)
                # t2 = min(left, right) for interior column strips
                nc.vector.tensor_tensor(
                    out=t2[:, EXT:LAST], in0=lab[:, 0:LAST - EXT],
                    in1=lab[:, 2 * EXT:FREE], op=mn).then_inc(dve_sem, 1)
                # boundary strips: c=0 (left from PE shift), c=15 (right)
                nc.vector.tensor_tensor(
                    out=t2[:, 0:EXT], in0=lb[:], in1=lab[:, EXT:2 * EXT],
                    op=mn).then_inc(dve_sem, 1)
                nc.vector.tensor_tensor(
                    out=t2[:, LAST:FREE], in0=lab[:, LAST - EXT:LAST],
                    in1=rb[:], op=mn).then_inc(dve_sem, 1)
                # t1 = min(t1, t2) ; edges of t1 (f=0, f=FREE-1) are halo rows
                nc.vector.tensor_tensor(
                    out=t1[:, 1:FREE - 1], in0=t1[:, 1:FREE - 1],
                    in1=t2[:, 1:FREE - 1], op=mn).then_inc(dve_sem, 1)
                # lab = max(min(t1, lab), mb)   (mb = BIG where masked out)
                nc.vector.tensor_tensor(
                    out=t2[:], in0=t1[:], in1=lab[:], op=mn)
                nc.vector.tensor_tensor(
                    out=lab[:], in0=t2[:], in1=mb[:], op=mx).then_inc(dve_sem, 1)
        # note: t1[*,0]/t1[*,FREE-1] keep stale values; they only affect the
        # extreme halo row of each strip which is discarded by construction.

    _CC_CACHE["nc"] = nc
    return nc


# ----------------------------------------------------------------------------
# host-side reference pieces (cascade conv runs here in V0; counting / top-16
# / per-ROI classifier are small and data-dependent)
# ----------------------------------------------------------------------------

def _conv3x3_same(x, w):
    Cin, Hh, Ww = x.shape
    Cout = w.shape[0]
    xp = np.zeros((Cin, Hh + 2, Ww + 2), np.float32)
    xp[:, 1:-1, 1:-1] = x
    out = np.zeros((Cout, Hh, Ww), np.float32)
    for dy in range(3):
        for dx in range(3):
            v = xp[:, dy:dy + Hh, dx:dx + Ww].reshape(Cin, -1)
            out += (w[:, :, dy, dx] @ v).reshape(Cout, Hh, Ww)
    return out


def _cascade_logits(x, w1, w2, w3):
    h = np.maximum(_conv3x3_same(x[0], w1), 0.0)
    h = np.maximum(_conv3x3_same(h, w2), 0.0)
    return np.einsum('oi,ihw->ohw', w3[:, :, 0, 0], h)[0]


def _roi_resize(img, y0, y1, x0, x1):
    t = np.linspace(0.0, 1.0, CS, dtype=np.float32)
    gy = np.float32(y0) + t * np.float32(y1 - y0)
    gx = np.float32(x0) + t * np.float32(x1 - x0)
    yf = np.clip(np.floor(gy).astype(np.int32), 0, H - 2)
    xf = np.clip(np.floor(gx).astype(np.int32), 0, W - 2)
    fy = (gy - yf)[:, None].astype(np.float32)
    fx = (gx - xf)[None, :].astype(np.float32)
    v00 = img[yf[:, None], xf[None, :]]
    v01 = img[yf[:, None], (xf + 1)[None, :]]
    v10 = img[(yf + 1)[:, None], xf[None, :]]
    v11 = img[(yf + 1)[:, None], (xf + 1)[None, :]]
    return (1 - fy) * ((1 - fx) * v00 + fx * v01) + fy * ((1 - fx) * v10 + fx * v11)


def _classifier(roi, cw1, cw2, fc):
    O1 = (CS - 7) // 4 + 1
    pat = np.lib.stride_tricks.sliding_window_view(roi, (7, 7))[::4, ::4]
    h1 = np.maximum(np.einsum('oyx,abyx->oab', cw1[:, 0], pat), 0.0)
    pat2 = np.lib.stride_tricks.sliding_window_view(h1, (3, 3), axis=(1, 2))[:, ::2, ::2]
    h2 = np.maximum(np.einsum('oiyx,iabyx->oab', cw2, pat2), 0.0)
    return h2.mean(axis=(1, 2)) @ fc


def kernel(x, w1, w2, w3, cw1, cw2, fc):
    x = np.asarray(x, np.float32)
    w1 = np.asarray(w1, np.float32); w2 = np.asarray(w2, np.float32)
    w3 = np.asarray(w3, np.float32)
    cw1 = np.asarray(cw1, np.float32); cw2 = np.asarray(cw2, np.float32)
    fc = np.asarray(fc, np.float32)

    # cascade -> logits -> mask  (host in V0; z > 0 <=> sigmoid(z) > 0.5)
    z = _cascade_logits(x, w1, w2, w3)
    mask = z > 0.0

    # per-core CC inputs in the [128, FREE] column-strip layout
    iota = np.arange(H * W, dtype=np.float32).reshape(H, W)
    lab0_full = np.where(mask, iota, BIG).astype(np.float32)
    mb_full = np.where(mask, 0.0, BIG).astype(np.float32)

    sdn = np.zeros((128, 128), np.float32)
    sdn[np.arange(127), np.arange(1, 128)] = 1.0     # Sdn[k, k+1] = 1
    sup = np.zeros((128, 128), np.float32)
    sup[np.arange(1, 128), np.arange(127)] = 1.0     # Sup[k, k-1] = 1

    def to_cc(full, d):
        r0 = 256 * d - HALO
        ext = np.full((EXT, W), BIG, np.float32)
        lo, hi = max(r0, 0), min(r0 + EXT, H)
        ext[lo - r0:hi - r0] = full[lo:hi]
        return np.ascontiguousarray(
            ext.reshape(EXT, 128, CSTRIP).transpose(1, 2, 0).reshape(128, FREE))

    biasl = np.zeros((128, 1), np.float32); biasl[0, 0] = BIG
    biasr = np.zeros((128, 1), np.float32); biasr[127, 0] = BIG
    in_maps = []
    for d in range(NCORES):
        in_maps.append({
            "lab0": to_cc(lab0_full, d),
            "mb": to_cc(mb_full, d),
            "sdn": sdn,
            "sup": sup,
            "biasl": biasl,
            "biasr": biasr,
        })

    global _LAST_IN_MAPS, _LAST_DEVICE_WALL
    _LAST_IN_MAPS = in_maps
    nc = _build_cc_program()
    import time as _time
    _t0 = _time.time()
    res = run_bass_kernel_spmd(nc, in_maps, list(range(NCORES)))
    _LAST_DEVICE_WALL = _time.time() - _t0
    lab = np.concatenate(
        [res.results[d]["lab_out"].reshape(128, CSTRIP, OWN)
         .transpose(2, 0, 1).reshape(OWN, W) for d in range(NCORES)],
        axis=0).astype(np.int64)

    # counts + stable top-16 (jax.lax.top_k: descending, ties -> lower index)
    counts = np.bincount(lab.ravel(), minlength=H * W + 1)
    counts[H * W] = 0
    lids = np.argsort(-counts.astype(np.int64), kind='stable')[:K]
    cnts = counts[lids]

    img = x[0, 0]
    out = np.zeros((NC_CLS, H, W), np.float32)
    remap = {}
    for k in range(K):
        lid, cnt = int(lids[k]), int(cnts[k])
        cm = lab == lid
        rowany = cm.any(axis=1)
        colany = cm.any(axis=0)
        y0 = int(np.argmax(rowany)); y1 = H - 1 - int(np.argmax(rowany[::-1]))
        x0 = int(np.argmax(colany)); x1 = W - 1 - int(np.argmax(colany[::-1]))
        roi = _roi_resize(img, y0, y1, x0, x1)
        lg = _classifier(roi, cw1, cw2, fc)
        cls = int(np.argmax(lg))
        if cnt >= MIN_SIZE:
            out[cls] = np.maximum(out[cls], cm.astype(np.float32))
    return np.clip(out, 0.0, 1.0)


if __name__ == "__main__":
    ins = {k: np.load(f"/tmp/in_{k}.npy") for k in
           ["x", "w1", "w2", "w3", "cw1", "cw2", "fc"]}
    out = kernel(**ins)
    ref = np.load("/tmp/np_out.npy")
    print("max abs err vs np_ref:", np.abs(out - ref).max())


# revision 26
# speedup vs baseline: 1.1496x; 1.1496x over previous
"""Trainium2 Bass kernel for nn_DetectNet (nms_detection).

Sharding: data-parallel over row-bands — core d owns frame rows
[256d, 256(d+1)). The 64-iteration connected-components label propagation
runs fully on-device on all 8 NeuronCores with 64-row halos per side
(zero cross-core communication: error from the halo edge travels 1 row per
iteration, so owned rows stay exact).

Per-core CC layout: [128 partitions, 6144 free], partition p holds frame
columns [16p, 16p+16), free index = c_local*384 + r (column strips of 16,
row-major within strip). Row-neighbor min = free-offset +-1; column-neighbor
min = free-offset +-384 (within partition) plus a cross-partition shift of the
boundary strips done on the TensorEngine via a 128x128 shift matrix.

V0 scope: the 64 CC propagation iterations (the dominant, DVE-bound stage)
run on the 8 NeuronCores via run_bass_kernel_spmd; the conv cascade and the
small data-dependent tail (4M-bin bincount -> stable top-16, per-component
bbox/ROI/classifier, paint) run on the host. Raw Block-based Bass is used
throughout because TileContext-emitted BIR fails walrus codegen
(setupSyncWait) in this environment.
"""
import numpy as np

import concourse.bass as bass
import concourse.mybir as mybir
from concourse.bass_utils import run_bass_kernel_spmd

H, W = 2048, 2048
CS = 227
K = 16
NC_CLS = 4
MIN_SIZE = 64
CC_ITERS = 64
BIG = float(H * W)

NCORES = 8
OWN = H // NCORES          # 256 rows owned per core
HALO = 64
EXT = OWN + 2 * HALO       # 384 rows in CC array
CSTRIP = 16                # columns per partition
FREE = CSTRIP * EXT        # 6144


# ----------------------------------------------------------------------------
# device program: 64 CC iterations (min-label propagation, 4-connectivity)
# ----------------------------------------------------------------------------
_CC_CACHE = {}


def _build_cc_program():
    if "nc" in _CC_CACHE:
        return _CC_CACHE["nc"]
    dt = mybir.dt.float32
    nc = bass.Bass()
    lab_in = nc.dram_tensor("lab0", [128, FREE], dt, kind="ExternalInput")
    mb_in = nc.dram_tensor("mb", [128, FREE], dt, kind="ExternalInput")
    # shift matrices: Sdn[k, m] = 1 if k == m-1 ; Sup[k, m] = 1 if k == m+1
    sdn_in = nc.dram_tensor("sdn", [128, 128], dt, kind="ExternalInput")
    sup_in = nc.dram_tensor("sup", [128, 128], dt, kind="ExternalInput")
    # per-partition bias vectors: BIG at the frame-edge partition, else 0
    bl_in = nc.dram_tensor("biasl", [128, 1], dt, kind="ExternalInput")
    br_in = nc.dram_tensor("biasr", [128, 1], dt, kind="ExternalInput")
    lab_out = nc.dram_tensor("lab_out", [128, CSTRIP * OWN], dt,
                             kind="ExternalOutput")

    with (
        nc.sbuf_tensor([128, FREE], dt) as lab,
        nc.sbuf_tensor([128, FREE], dt) as mb,
        nc.sbuf_tensor([128, FREE], dt) as t1,
        nc.sbuf_tensor([128, FREE], dt) as t2,
        nc.sbuf_tensor([128, 128], dt) as sdn,
        nc.sbuf_tensor([128, 128], dt) as sup,
        nc.sbuf_tensor([128, 1], dt) as bl,
        nc.sbuf_tensor([128, 1], dt) as br,
        nc.sbuf_tensor([128, EXT], dt) as lb,       # left-neighbor strip
        nc.sbuf_tensor([128, EXT], dt) as rb,       # right-neighbor strip
        nc.psum_tensor([128, EXT], dt) as psl,
        nc.psum_tensor([128, EXT], dt) as psr,
        nc.semaphore("dma_sem") as dma_sem,
        nc.semaphore("pe_sem") as pe_sem,
        nc.semaphore("act_sem") as act_sem,
        nc.semaphore("dve_sem") as dve_sem,
        nc.Block() as block,
    ):
        mn = mybir.AluOpType.min
        mx = mybir.AluOpType.max
        DVE_PER = 6   # dve sem increments per iteration
        LAST = EXT * (CSTRIP - 1)  # free offset of last column strip = 5760

        @block.sync
        def _(sync):
            sync.dma_start(out=lab[:], in_=lab_in[:]).then_inc(dma_sem, 16)
            sync.dma_start(out=mb[:], in_=mb_in[:]).then_inc(dma_sem, 16)
            sync.dma_start(out=sdn[:], in_=sdn_in[:]).then_inc(dma_sem, 16)
            sync.dma_start(out=sup[:], in_=sup_in[:]).then_inc(dma_sem, 16)
            sync.dma_start(out=bl[:], in_=bl_in[:]).then_inc(dma_sem, 16)
            sync.dma_start(out=br[:], in_=br_in[:]).then_inc(dma_sem, 16)
            # wait for the full CC loop to finish, then write owned rows out
            sync.wait_ge(dve_sem, CC_ITERS * DVE_PER)
            src = lab[:].rearrange("p (c r) -> p c r", r=EXT)[:, :, HALO:HALO + OWN]
            dst = lab_out[:].rearrange("p (c r) -> p c r", r=OWN)
            sync.dma_start(out=dst, in_=src).then_inc(dma_sem, 16)

        @block.tensor
        def _(tensor):
            tensor.wait_ge(dma_sem, 96)
            for i in range(CC_ITERS):
                # wait for lab from previous iteration (op 8 of iter i)
                if i > 0:
                    tensor.wait_ge(dve_sem, i * DVE_PER)
                # psl[p] = lab[p-1, last strip]  (left neighbor of col strip 0)
                nc.tensor.matmul(out=psl[:], lhsT=sdn[:],
                                 rhs=lab[:, LAST:LAST + EXT],
                                 start=True, stop=True).then_inc(pe_sem, 1)
                # psr[p] = lab[p+1, strip 0]  (right neighbor of last strip)
                nc.tensor.matmul(out=psr[:], lhsT=sup[:],
                                 rhs=lab[:, 0:EXT],
                                 start=True, stop=True).then_inc(pe_sem, 1)

        @block.scalar
        def _(scalar):
            cp = mybir.ActivationFunctionType.Identity
            scalar.wait_ge(dma_sem, 96)
            for i in range(CC_ITERS):
                scalar.wait_ge(pe_sem, i * 2 + 2)
                # copy PSUM->SBUF, adding BIG on the frame-edge partition so
                # the shifted-in "neighbor" is neutral in the min
                nc.scalar.activation(out=lb[:], in_=psl[:], func=cp,
                                     bias=bl[:, 0:1]).then_inc(act_sem, 1)
                nc.scalar.activation(out=rb[:], in_=psr[:], func=cp,
                                     bias=br[:, 0:1]).then_inc(act_sem, 1)

        @block.vector
        def _(vector):
            vector.wait_ge(dma_sem, 96)
            for i in range(CC_ITERS):
                vector.wait_ge(act_sem, i * 2 + 2)
                # t1 = min(up, down) along rows (free offset +-1)
                nc.vector.tensor_tensor(
                    out=t1[:, 1:FREE - 1], in0=lab[:, 0:FREE - 2],
                    in1=lab[:, 2:FREE], op=mn).then_inc(dve_sem, 1)
                # t2 = min(left, right) for interior column strips
                nc.vector.tensor_tensor(
                    out=t2[:, EXT:LAST], in0=lab[:, 0:LAST - EXT],
                    in1=lab[:, 2 * EXT:FREE], op=mn).then_inc(dve_sem, 1)
                # boundary strips: c=0 (left from PE shift), c=15 (right)
                nc.vector.tensor_tensor(
                    out=t2[:, 0:EXT], in0=lb[:], in1=lab[:, EXT:2 * EXT],
                    op=mn).then_inc(dve_sem, 1)
                nc.vector.tensor_tensor(
                    out=t2[:, LAST:FREE], in0=lab[:, LAST - EXT:LAST],
                    in1=rb[:], op=mn).then_inc(dve_sem, 1)
                # t1 = min(t1, t2) ; edges of t1 (f=0, f=FREE-1) are halo rows
                nc.vector.tensor_tensor(
                    out=t1[:, 1:FREE - 1], in0=t1[:, 1:FREE - 1],
                    in1=t2[:, 1:FREE - 1], op=mn).then_inc(dve_sem, 1)
                # lab = max(min(t1, lab), mb)   (mb = BIG where masked out)
                nc.vector.tensor_tensor(
                    out=t2[:], in0=t1[:], in1=lab[:], op=mn)
                nc.vector.tensor_tensor(
                    out=lab[:], in0=t2[:], in1=mb[:], op=mx).then_inc(dve_sem, 1)
        # note: t1[*,0]/t1[*,FREE-1] keep stale values; they only affect the
        # extreme halo row of each strip which is discarded by construction.

    _CC_CACHE["nc"] = nc
    return nc


# ----------------------------------------------------------------------------
# device program 2: conv cascade -> pre-sigmoid logits z on the 384-row
# extended band per core. fp32 matmuls with row-packed contraction:
# conv1 K=18 (3 dx-shifted x copies x 6 rows), conv2 K=128+64 banded pieces,
# conv3 K=128 block-diagonal. Strictly chained semaphores (no pipelining).
# ----------------------------------------------------------------------------
XW = W + 2          # 2050, padded cols
NT = EXT // 4 + 1   # 97 A-tiles (h1 rows 4t-1 .. 4t+2)
NG = EXT // 4       # 96 output groups of 4 rows


def _build_cascade_program():
    if "casc" in _CC_CACHE:
        return _CC_CACHE["casc"]
    dt = mybir.dt.float32
    nc = bass.Bass()
    x_in = nc.dram_tensor("xext", [EXT + 6, XW], dt, kind="ExternalInput")
    w1_in = nc.dram_tensor("w1s", [18, 128], dt, kind="ExternalInput")
    w2a_in = nc.dram_tensor("w2a", [128, 3, 128], dt, kind="ExternalInput")
    w2b_in = nc.dram_tensor("w2b", [64, 3, 128], dt, kind="ExternalInput")
    w3_in = nc.dram_tensor("w3s", [128, 4], dt, kind="ExternalInput")
    z_out = nc.dram_tensor("zout", [EXT, W], dt, kind="ExternalOutput")

    with (
        nc.sbuf_tensor([18, XW - 2], dt) as xs0,
        nc.sbuf_tensor([18, XW - 2], dt) as xs1,
        nc.sbuf_tensor([128, XW], dt) as a0,
        nc.sbuf_tensor([128, XW], dt) as a1,
        nc.sbuf_tensor([128, XW], dt) as a2,
        nc.sbuf_tensor([128, 512], dt) as h2,
        nc.sbuf_tensor([4, 512], dt) as zs,
        nc.sbuf_tensor([18, 128], dt) as w1s,
        nc.sbuf_tensor([128, 3, 128], dt) as w2a,
        nc.sbuf_tensor([64, 3, 128], dt) as w2b,
        nc.sbuf_tensor([128, 4], dt) as w3s,
        nc.psum_tensor([128, 512], dt) as ps1,
        nc.psum_tensor([128, 512], dt) as ps2,
        nc.psum_tensor([4, 512], dt) as ps3,
        nc.semaphore("dma_sem") as dma_sem,
        nc.semaphore("pe_sem") as pe_sem,
        nc.semaphore("act_sem") as act_sem,
        nc.semaphore("zd_sem") as zd_sem,
        nc.Block() as block,
    ):
        xs = [xs0, xs1]
        A = [a0, a1, a2]
        relu = mybir.ActivationFunctionType.Relu
        NXT = 4   # xtiles
        # PE op order: per t: [4 conv1 MMs]; then if t >= 1, g = t-1:
        #   per xtile: [6 conv2 MMs, 1 conv3 MM]
        # ACT op order: per t: [4 psum1->A relu copies]; if t>=1: [4 h2 copies]
        # sync order: 4 weight DMAs, per t: 3 xs DMAs.  z DMAs on gpsimd.

        # replay the schedules to precompute wait counts
        PE_C1_END = {}
        PE_C3_END = {}
        c = 0
        for t in range(NT):
            c += NXT
            PE_C1_END[t] = c
            if t >= 1:
                g = t - 1
                for xt in range(NXT):
                    c += 7
                    PE_C3_END[g * NXT + xt] = c
        ACT_A_END = {}
        ACT_H2_END = {}
        ACT_Z_END = {}
        a = 0
        for t in range(NT):
            a += NXT
            ACT_A_END[t] = a
            if t >= 1:
                g = t - 1
                for xt in range(NXT):
                    a += 1
                    ACT_H2_END[g * NXT + xt] = a
                    a += 1
                    ACT_Z_END[g * NXT + xt] = a

        @block.sync
        def _(sync):
            for wt, wi in ((w1s, w1_in), (w2a, w2a_in), (w2b, w2b_in),
                           (w3s, w3_in)):
                sync.dma_start(out=wt[:], in_=wi[:]).then_inc(dma_sem, 16)
            for t in range(NT):
                # xs double buffer: reuse slot t%2 -> wait conv1 of t-2 done
                if t >= 2:
                    sync.wait_ge(pe_sem, PE_C1_END[t - 2])
                for s in range(3):
                    sync.dma_start(out=xs[t % 2][6 * s:6 * s + 6, :],
                                   in_=x_in[4 * t:4 * t + 6, s:s + W]
                                   ).then_inc(dma_sem, 16)

        @block.gpsimd
        def _(gpsimd):
            for g in range(NG):
                for xt in range(NXT):
                    gpsimd.wait_ge(act_sem, ACT_Z_END[g * NXT + xt])
                    gpsimd.dma_start(
                        out=z_out[4 * g:4 * g + 4, 512 * xt:512 * xt + 512],
                        in_=zs[:, :]).then_inc(zd_sem, 16)

        @block.tensor
        def _(tensor):
            tensor.wait_ge(dma_sem, 64)     # weights
            for t in range(NT):
                # xs for tile t ready after (4 + 3*(t+1)) DMAs
                tensor.wait_ge(dma_sem, 16 * (4 + 3 * (t + 1)))
                for xt in range(NXT):
                    # ps1 reuse: wait for ACT A-copy of previous use
                    if t >= 1 and xt == 0:
                        tensor.wait_ge(act_sem, ACT_A_END[t - 1])
                    elif xt > 0:
                        tensor.wait_ge(act_sem, ACT_A_END[t] - NXT + xt)
                    nc.tensor.matmul(
                        out=ps1[:], lhsT=w1s[:],
                        rhs=xs[t % 2][:, 512 * xt:512 * xt + 512],
                        start=True, stop=True).then_inc(pe_sem, 1)
                if t >= 1:
                    g = t - 1
                    # A_g, A_{g+1} fully written by ACT
                    tensor.wait_ge(act_sem, ACT_A_END[t])
                    for xt in range(NXT):
                        # ps2 reuse: wait h2 copy of previous (g, xt)
                        k = g * NXT + xt
                        if k >= 1:
                            tensor.wait_ge(act_sem, ACT_H2_END[k - 1])
                        for dx in range(3):
                            nc.tensor.matmul(
                                out=ps2[:], lhsT=w2a[:, dx, :],
                                rhs=A[g % 3][:, 512 * xt + dx:512 * xt + dx + 512],
                                start=(dx == 0), stop=False).then_inc(pe_sem, 1)
                            nc.tensor.matmul(
                                out=ps2[:], lhsT=w2b[:, dx, :],
                                rhs=A[(g + 1) % 3][0:64, 512 * xt + dx:512 * xt + dx + 512],
                                start=False, stop=(dx == 2)).then_inc(pe_sem, 1)
                        # conv3: wait h2 written; ps3 drained by z-copy k-1
                        tensor.wait_ge(act_sem, ACT_H2_END[k])
                        if k >= 1:
                            tensor.wait_ge(act_sem, ACT_Z_END[k - 1])
                        nc.tensor.matmul(
                            out=ps3[:], lhsT=w3s[:], rhs=h2[:],
                            start=True, stop=True).then_inc(pe_sem, 1)

        @block.scalar
        def _(scalar):
            for t in range(NT):
                for xt in range(NXT):
                    scalar.wait_ge(pe_sem, PE_C1_END[t] - NXT + xt + 1)
                    nc.scalar.activation(
                        out=A[(t % 3)][:, 512 * xt + 1:512 * xt + 513],
                        in_=ps1[:], func=relu).then_inc(act_sem, 1)
                if t >= 1:
                    g = t - 1
                    idf = mybir.ActivationFunctionType.Identity
                    for xt in range(NXT):
                        k = g * NXT + xt
                        scalar.wait_ge(pe_sem, PE_C3_END[k] - 1)
                        nc.scalar.activation(
                            out=h2[:], in_=ps2[:], func=relu).then_inc(act_sem, 1)
                        # z staging: wait conv3 result and z-DMA drain of zs
                        scalar.wait_ge(pe_sem, PE_C3_END[k])
                        if k >= 1:
                            scalar.wait_ge(zd_sem, 16 * k)
                        nc.scalar.activation(
                            out=zs[:], in_=ps3[:], func=idf).then_inc(act_sem, 1)

        @block.vector
        def _(vector):
            # zero the padded edge columns of the A tiles once at startup;
            # ACT only ever writes cols 1..2048, conv2 reads 0..2049.
            for buf in A:
                nc.vector.memset(buf[:, 0:1], 0.0)
                nc.vector.memset(buf[:, XW - 1:XW], 0.0)

    _CC_CACHE["casc"] = nc
    return nc


def _prep_cascade_weights(w1, w2, w3):
    w1s = np.zeros((18, 128), np.float32)
    for s in range(3):
        for r in range(6):
            for j in range(4):
                dy = r - j
                if 0 <= dy <= 2:
                    w1s[s * 6 + r, j * 32:j * 32 + 32] = w1[:, 0, dy, s]
    w2a = np.zeros((128, 3, 128), np.float32)
    w2b = np.zeros((64, 3, 128), np.float32)
    for dx in range(3):
        for b in range(4):
            for j in range(4):
                dy = b - j
                if 0 <= dy <= 2:
                    w2a[b * 32:b * 32 + 32, dx, j * 32:j * 32 + 32] = \
                        w2[:, :, dy, dx].T
        for b2 in range(2):
            for j in range(4):
                dy = 4 + b2 - j
                if 0 <= dy <= 2:
                    w2b[b2 * 32:b2 * 32 + 32, dx, j * 32:j * 32 + 32] = \
                        w2[:, :, dy, dx].T
    w3s = np.zeros((128, 4), np.float32)
    for j in range(4):
        w3s[j * 32:j * 32 + 32, j] = w3[0, :, 0, 0]
    return w1s, w2a, w2b, w3s


def _device_cascade(x, w1, w2, w3):
    """Returns full-frame z (pre-sigmoid logits) rows computed per core on
    the extended band; caller slices owned rows."""
    w1s, w2a, w2b, w3s = _prep_cascade_weights(w1, w2, w3)
    img = x[0, 0]
    in_maps = []
    for d in range(NCORES):
        r0 = 256 * d - HALO - 2            # x rows needed: ext -2 .. 387
        xe = np.zeros((EXT + 6, XW), np.float32)
        lo, hi = max(r0, 0), min(r0 + EXT + 6, H)
        xe[lo - r0:hi - r0, 1:1 + W] = img[lo:hi]
        in_maps.append({"xext": xe, "w1s": w1s, "w2a": w2a, "w2b": w2b,
                        "w3s": w3s})
    nc = _build_cascade_program()
    res = run_bass_kernel_spmd(nc, in_maps, list(range(NCORES)))
    # stitch owned rows: core d's z rows HALO..HALO+OWN are frame rows 256d..
    z = np.empty((H, W), np.float32)
    for d in range(NCORES):
        z[256 * d:256 * (d + 1)] = res.results[d]["zout"][HALO:HALO + OWN]
    # frame rows 0 / 2047: reference zero-pads h1 at the frame edge, while the
    # device band computes real conv1 values there; patch the two edge rows.
    h1t = np.maximum(_conv3x3_same(x[0, :, 0:4, :], w1), 0.0)
    h1b = np.maximum(_conv3x3_same(x[0, :, H - 4:H, :], w1), 0.0)
    zt = np.einsum('oi,ihw->ohw', w3[:, :, 0, 0],
                   np.maximum(_conv3x3_same(h1t, w2), 0.0))[0]
    zb = np.einsum('oi,ihw->ohw', w3[:, :, 0, 0],
                   np.maximum(_conv3x3_same(h1b, w2), 0.0))[0]
    z[0] = zt[0]
    z[H - 1] = zb[-1]
    return z


# ----------------------------------------------------------------------------
# host-side reference pieces (counting / top-16 / per-ROI classifier are
# small and data-dependent; numpy cascade kept as fallback)
# ----------------------------------------------------------------------------

def _conv3x3_same(x, w):
    Cin, Hh, Ww = x.shape
    Cout = w.shape[0]
    xp = np.zeros((Cin, Hh + 2, Ww + 2), np.float32)
    xp[:, 1:-1, 1:-1] = x
    out = np.zeros((Cout, Hh, Ww), np.float32)
    for dy in range(3):
        for dx in range(3):
            v = xp[:, dy:dy + Hh, dx:dx + Ww].reshape(Cin, -1)
            out += (w[:, :, dy, dx] @ v).reshape(Cout, Hh, Ww)
    return out


def _cascade_logits(x, w1, w2, w3):
    h = np.maximum(_conv3x3_same(x[0], w1), 0.0)
    h = np.maximum(_conv3x3_same(h, w2), 0.0)
    return np.einsum('oi,ihw->ohw', w3[:, :, 0, 0], h)[0]


def _roi_resize(img, y0, y1, x0, x1):
    t = np.linspace(0.0, 1.0, CS, dtype=np.float32)
    gy = np.float32(y0) + t * np.float32(y1 - y0)
    gx = np.float32(x0) + t * np.float32(x1 - x0)
    yf = np.clip(np.floor(gy).astype(np.int32), 0, H - 2)
    xf = np.clip(np.floor(gx).astype(np.int32), 0, W - 2)
    fy = (gy - yf)[:, None].astype(np.float32)
    fx = (gx - xf)[None, :].astype(np.float32)
    v00 = img[yf[:, None], xf[None, :]]
    v01 = img[yf[:, None], (xf + 1)[None, :]]
    v10 = img[(yf + 1)[:, None], xf[None, :]]
    v11 = img[(yf + 1)[:, None], (xf + 1)[None, :]]
    return (1 - fy) * ((1 - fx) * v00 + fx * v01) + fy * ((1 - fx) * v10 + fx * v11)


def _classifier(roi, cw1, cw2, fc):
    O1 = (CS - 7) // 4 + 1
    pat = np.lib.stride_tricks.sliding_window_view(roi, (7, 7))[::4, ::4]
    h1 = np.maximum(np.einsum('oyx,abyx->oab', cw1[:, 0], pat), 0.0)
    pat2 = np.lib.stride_tricks.sliding_window_view(h1, (3, 3), axis=(1, 2))[:, ::2, ::2]
    h2 = np.maximum(np.einsum('oiyx,iabyx->oab', cw2, pat2), 0.0)
    return h2.mean(axis=(1, 2)) @ fc


def kernel(x, w1, w2, w3, cw1, cw2, fc):
    x = np.asarray(x, np.float32)
    w1 = np.asarray(w1, np.float32); w2 = np.asarray(w2, np.float32)
    w3 = np.asarray(w3, np.float32)
    cw1 = np.asarray(cw1, np.float32); cw2 = np.asarray(cw2, np.float32)
    fc = np.asarray(fc, np.float32)

    # cascade -> logits -> mask  (z > 0 <=> sigmoid(z) > 0.5)
    try:
        z = _device_cascade(x, w1, w2, w3)
    except Exception as e:
        print("device cascade failed, falling back to host:", str(e)[:200])
        z = _cascade_logits(x, w1, w2, w3)
    mask = z > 0.0

    # per-core CC inputs in the [128, FREE] column-strip layout
    iota = np.arange(H * W, dtype=np.float32).reshape(H, W)
    lab0_full = np.where(mask, iota, BIG).astype(np.float32)
    mb_full = np.where(mask, 0.0, BIG).astype(np.float32)

    sdn = np.zeros((128, 128), np.float32)
    sdn[np.arange(127), np.arange(1, 128)] = 1.0     # Sdn[k, k+1] = 1
    sup = np.zeros((128, 128), np.float32)
    sup[np.arange(1, 128), np.arange(127)] = 1.0     # Sup[k, k-1] = 1

    def to_cc(full, d):
        r0 = 256 * d - HALO
        ext = np.full((EXT, W), BIG, np.float32)
        lo, hi = max(r0, 0), min(r0 + EXT, H)
        ext[lo - r0:hi - r0] = full[lo:hi]
        return np.ascontiguousarray(
            ext.reshape(EXT, 128, CSTRIP).transpose(1, 2, 0).reshape(128, FREE))

    biasl = np.zeros((128, 1), np.float32); biasl[0, 0] = BIG
    biasr = np.zeros((128, 1), np.float32); biasr[127, 0] = BIG
    in_maps = []
    for d in range(NCORES):
        in_maps.append({
            "lab0": to_cc(lab0_full, d),
            "mb": to_cc(mb_full, d),
            "sdn": sdn,
            "sup": sup,
            "biasl": biasl,
            "biasr": biasr,
        })

    global _LAST_IN_MAPS, _LAST_DEVICE_WALL
    _LAST_IN_MAPS = in_maps
    nc = _build_cc_program()
    import time as _time
    _t0 = _time.time()
    res = run_bass_kernel_spmd(nc, in_maps, list(range(NCORES)))
    _LAST_DEVICE_WALL = _time.time() - _t0
    lab = np.concatenate(
        [res.results[d]["lab_out"].reshape(128, CSTRIP, OWN)
         .transpose(2, 0, 1).reshape(OWN, W) for d in range(NCORES)],
        axis=0).astype(np.int64)

    # counts + stable top-16 (jax.lax.top_k: descending, ties -> lower index)
    counts = np.bincount(lab.ravel(), minlength=H * W + 1)
    counts[H * W] = 0
    lids = np.argsort(-counts.astype(np.int64), kind='stable')[:K]
    cnts = counts[lids]

    img = x[0, 0]
    out = np.zeros((NC_CLS, H, W), np.float32)
    remap = {}
    for k in range(K):
        lid, cnt = int(lids[k]), int(cnts[k])
        cm = lab == lid
        rowany = cm.any(axis=1)
        colany = cm.any(axis=0)
        y0 = int(np.argmax(rowany)); y1 = H - 1 - int(np.argmax(rowany[::-1]))
        x0 = int(np.argmax(colany)); x1 = W - 1 - int(np.argmax(colany[::-1]))
        roi = _roi_resize(img, y0, y1, x0, x1)
        lg = _classifier(roi, cw1, cw2, fc)
        cls = int(np.argmax(lg))
        if cnt >= MIN_SIZE:
            out[cls] = np.maximum(out[cls], cm.astype(np.float32))
    return np.clip(out, 0.0, 1.0)


if __name__ == "__main__":
    ins = {k: np.load(f"/tmp/in_{k}.npy") for k in
           ["x", "w1", "w2", "w3", "cw1", "cw2", "fc"]}
    out = kernel(**ins)
    ref = np.load("/tmp/np_out.npy")
    print("max abs err vs np_ref:", np.abs(out - ref).max())
